# revision 26
# baseline (speedup 1.0000x reference)
"""Trainium2 Bass kernel: 16-head causal attention with sink logit.

Contract: kernel(**inputs) takes the FULL inputs of the reference
(x [2,2048,1024], W_Q/W_K/W_V/W_out [1024,1024], sink [16]) and returns
the FULL output [2,2048,1024], running on 8 NeuronCores.

Sharding: core c = b*4 + g handles batch b and heads [4g, 4g+4).
Each core computes yT_partial [1024, 2048] = W_out_slice^T @ attn^T;
host sums the 4 partials per batch and transposes.

v2 structure: single software-pipelined stream. Projections (K^T, V,
Q^T per 512-token chunk), W_out and normalize blocks are emitted as
"filler" closures popped between attention tile steps so the PE stays
busy while the scalar engine runs exp. Diagonal score tiles are
narrowed to the valid q-range; only the 128x128 triangle block is
masked. Denominators are broadcast raw via one PE matmul and the
normalization is a DVE divide reading PSUM directly.
"""

import sys
import numpy as np

if "/opt/trn_rl_repo" not in sys.path:
    sys.path.insert(0, "/opt/trn_rl_repo")

B, T, C = 2, 2048, 1024
H, D = 16, 64
G = 4                # heads per core
DH = G * D           # 256 head-dims per core
NCORES = 8
QC = 512             # q chunk (matmul moving free dim)
NQ = T // QC         # 4
NKT = T // 128       # 16 k-tiles
NCC = C // 128       # 8 contraction chunks over C
SCALE = 1.0 / float(np.sqrt(D))

# vp_sb per-kt slot layout (386 cols per kt):
#   head0 (even): [V(64) | one]            off 0,   width 65,  denom row 64
#   head1 (odd):  [one | zeros(63) | V(64)] off 65,  width 128, denom row 0
#   head2 (even): [V(64) | one]            off 193, width 65,  denom row 64
#   head3 (odd):  [one | zeros(63) | V(64)] off 258, width 128, denom row 0
VP_W = 386
VP_OFF = [0, 65, 193, 258]
VSEG = [(0, 0, 64), (64, 129, 128), (192, 322, 64)]  # (psum off, vp off, width)


def build_program(reps=1):
    """Build the per-core Bass program. reps>1 repeats the compute body
    (same inputs -> same outputs) for differential wall-clock timing."""
    from contextlib import ExitStack

    import concourse.bass as bass
    import concourse.tile as tile
    from concourse import bacc, mybir

    f32 = mybir.dt.float32
    f32r = mybir.dt.float32r
    bf16 = mybir.dt.bfloat16
    AF = mybir.ActivationFunctionType
    Alu = mybir.AluOpType

    nc = bacc.Bacc("TRN2", target_bir_lowering=False, debug=False)

    xt_d = nc.dram_tensor("xt", [C, T], f32r, kind="ExternalInput").ap()
    wq_d = nc.dram_tensor("wq", [C, DH], f32r, kind="ExternalInput").ap()
    wk_d = nc.dram_tensor("wk", [C, DH], f32r, kind="ExternalInput").ap()
    wv_d = nc.dram_tensor("wv", [C, DH], f32r, kind="ExternalInput").ap()
    wo_d = nc.dram_tensor("wo", [DH, C], f32r, kind="ExternalInput").ap()
    sk_d = nc.dram_tensor("sk", [1, G], f32, kind="ExternalInput").ap()
    tri_d = nc.dram_tensor("tri", [128, 128], bf16, kind="ExternalInput").ap()
    vpc_d = nc.dram_tensor("vpc", [128, NKT * 65], f32r, kind="ExternalInput").ap()
    ind2_d = nc.dram_tensor("ind2", [128, 128], f32r, kind="ExternalInput").ap()
    dz_d = nc.dram_tensor("dz", [128, QC], f32r, kind="ExternalInput").ap()
    yt_d = nc.dram_tensor("yt", [C, T], bf16, kind="ExternalOutput").ap()

    xt_v = xt_d.rearrange("(n p) m -> p n m", p=128)   # [128, 8, 2048]
    wq_v = wq_d.rearrange("(n p) m -> p n m", p=128)   # [128, 8, 256]
    wk_v = wk_d.rearrange("(n p) m -> p n m", p=128)
    wv_v = wv_d.rearrange("(n p) m -> p n m", p=128)
    wo_v = wo_d.rearrange("(n p) m -> p n m", p=128)   # [128, 2, 1024]
    yt_v = yt_d.rearrange("(n p) m -> p n m", p=128)   # [128, 8, 2048]

    with tile.TileContext(nc) as tc, ExitStack() as ctx:
        P = lambda name, bufs: ctx.enter_context(tc.tile_pool(name=name, bufs=bufs))
        const_p = P("const", 1)
        big_p = P("big", 1)
        p_p = P("p", 4)
        y_p = P("y", 2)
        ps_p = ctx.enter_context(tc.tile_pool(name="ps", bufs=2, space="PSUM"))
        o_p = ctx.enter_context(tc.tile_pool(name="o", bufs=2, space="PSUM"))

        # ---- persistent SBUF tensors ----
        xt_sb = big_p.tile([128, NCC * T], f32r, tag="xt")           # 64KB/part
        wq_sb = big_p.tile([128, NCC * DH], f32r, tag="wq")
        wk_sb = big_p.tile([128, NCC * DH], f32r, tag="wk")
        wv_sb = big_p.tile([128, NCC * DH], f32r, tag="wv")
        wo_sb = big_p.tile([128, 2 * C], f32r, tag="wo")
        qt_sb = big_p.tile([128, 2 * T], f32r, tag="qt")
        kt_sb = big_p.tile([128, 2 * T], f32r, tag="kt")
        vp_sb = big_p.tile([128, NKT * VP_W], f32r, tag="vp")
        at_sb = big_p.tile([128, 2 * T], f32r, tag="at")             # attn^T normalized
        tri_sb = const_p.tile([128, 128], bf16, tag="tri")
        ind2_sb = const_p.tile([128, 128], f32r, tag="ind2")
        skr_sb = const_p.tile([128, G], f32, tag="skr")
        esk_sb = const_p.tile([128, G], f32, tag="esk")
        dn0_sb = const_p.tile([128, QC], f32r, tag="dn0")
        dn1_sb = const_p.tile([128, QC], f32r, tag="dn1")
        dn_sb = [dn0_sb, dn1_sb]

        xt_view = xt_sb[:].rearrange("p (n m) -> p n m", m=T)        # [128,8,2048]
        qt_view = qt_sb[:].rearrange("p (n m) -> p n m", m=T)        # [128,2,2048]
        kt_view = kt_sb[:].rearrange("p (n m) -> p n m", m=T)
        vp_view = vp_sb[:].rearrange("p (k w) -> p k w", w=VP_W)     # [128,16,386]

        # ---- phase 0: loads + constants (outside reps; ordered for the
        # pipeline: consts, wk, xt chunk0, wv, wq, xt c1, c2, wo, c3) ----
        nc.sync.dma_start(tri_sb[:, :], tri_d[:, :])
        nc.sync.dma_start(ind2_sb[:, :], ind2_d[:, :])
        nc.sync.dma_start(skr_sb[0:1, :], sk_d[:, :])
        nc.sync.dma_start(skr_sb[64:65, :], sk_d[:, :])
        nc.scalar.activation(esk_sb[0:1, :], skr_sb[0:1, :], AF.Exp)
        nc.scalar.activation(esk_sb[64:65, :], skr_sb[64:65, :], AF.Exp)
        for dn in dn_sb:
            nc.sync.dma_start(dn[:, :], dz_d[:, :])
        vpc_view = vpc_d.rearrange("p (k w) -> p k w", w=65)
        nc.sync.dma_start(vp_view[:, :, 64:129], vpc_view[:, :, :])
        nc.sync.dma_start(vp_view[:, :, 257:322], vpc_view[:, :, :])

        def dma_xt_chunk(c):
            # per-ci pieces so the first projection matmuls can chase the
            # DMA stream instead of waiting for the whole 2MB chunk
            for ci in range(NCC):
                nc.sync.dma_start(
                    xt_view[:, ci, c * QC:(c + 1) * QC],
                    xt_v[:, ci, c * QC:(c + 1) * QC])

        nc.sync.dma_start(
            wk_sb[:].rearrange("p (n m) -> p n m", m=DH), wk_v[:, :, :])
        dma_xt_chunk(0)
        nc.sync.dma_start(
            wv_sb[:].rearrange("p (n m) -> p n m", m=DH), wv_v[:, :, :])
        nc.sync.dma_start(
            wq_sb[:].rearrange("p (n m) -> p n m", m=DH), wq_v[:, :, :])
        dma_xt_chunk(1)
        dma_xt_chunk(2)
        nc.sync.dma_start(
            wo_sb[:].rearrange("p (n m) -> p n m", m=C), wo_v[:, :, :])
        dma_xt_chunk(3)

        # ---- emission helpers ----

        def proj_qk_chunk(w_sb, dst_view, cidx):
            """Project one 512-token chunk of Q^T or K^T: [2*128 d, 512 t]."""
            def emit():
                ps = ps_p.tile([128, 2 * QC], f32, tag="ps")
                for mt in range(2):
                    for ci in range(NCC):
                        nc.tensor.matmul(
                            ps[:, mt * QC:(mt + 1) * QC],
                            w_sb[:, ci * DH + mt * 128: ci * DH + (mt + 1) * 128],
                            xt_view[:, ci, cidx * QC:(cidx + 1) * QC],
                            start=(ci == 0), stop=(ci == NCC - 1))
                nc.vector.tensor_copy(
                    dst_view[:, :, cidx * QC:(cidx + 1) * QC],
                    ps[:].rearrange("p (n m) -> p n m", m=QC))
            return emit

        def proj_v_pair(cidx, pair):
            """V natural [t, d] for 2 of the 4 128-token tiles of chunk cidx."""
            def emit():
                ps = ps_p.tile([128, 2 * QC], f32, tag="ps")
                for s in range(2):
                    tt = cidx * 4 + pair * 2 + s
                    for ci in range(NCC):
                        nc.tensor.matmul(
                            ps[:, s * DH:(s + 1) * DH],
                            xt_view[:, ci, tt * 128:(tt + 1) * 128],
                            wv_sb[:, ci * DH:(ci + 1) * DH],
                            start=(ci == 0), stop=(ci == NCC - 1))
                ps_v = ps[:].rearrange("p (n m) -> p n m", m=DH)
                tt0 = cidx * 4 + pair * 2
                for so, do, w in VSEG:
                    nc.vector.tensor_copy(
                        vp_view[:, tt0:tt0 + 2, do:do + w],
                        ps_v[:, 0:2, so:so + w])
            return emit

        def emit_scores(qc, p, kt):
            """Scores + exp (+ triangle mask on diag) for one (p, kt) tile.
            Returns (pexp, off) for the later PV step."""
            diag = kt - 4 * qc
            off = 0 if diag < 0 else 128 * diag
            ps = ps_p.tile([128, 2 * QC], f32, tag="ps")
            nc.tensor.matmul(
                ps[:, off:QC],
                kt_sb[0:64, p * T + kt * 128:p * T + (kt + 1) * 128],
                qt_sb[0:64, p * T + qc * QC + off:p * T + qc * QC + QC],
                start=True, stop=True)
            nc.tensor.matmul(
                ps[:, QC + off:2 * QC],
                kt_sb[64:128, p * T + kt * 128:p * T + (kt + 1) * 128],
                qt_sb[64:128, p * T + qc * QC + off:p * T + qc * QC + QC],
                start=True, stop=True)
            pexp = p_p.tile([128, 2 * QC], f32r, tag="p")
            ps_h = ps[:].rearrange("p (n m) -> p n m", m=QC)
            pe_h = pexp[:].rearrange("p (n m) -> p n m", m=QC)
            nc.scalar.activation(
                pe_h[:, :, off:QC], ps_h[:, :, off:QC], AF.Exp, scale=SCALE)
            if diag >= 0:
                with nc.allow_low_precision(reason="0/1 mask mult"):
                    nc.gpsimd.tensor_mul(
                        pexp[:, off:off + 128], pexp[:, off:off + 128],
                        tri_sb[:, :])
                    nc.vector.tensor_mul(
                        pexp[:, QC + off:QC + off + 128],
                        pexp[:, QC + off:QC + off + 128], tri_sb[:, :])
            return pexp, off

        def emit_pv(qc, p, kt, nkt, oAB, pexp, off):
            base = kt * VP_W
            hA = VP_OFF[2 * p]
            hB = VP_OFF[2 * p + 1]
            nc.tensor.matmul(
                oAB[0:65, off:QC],
                vp_sb[:, base + hA:base + hA + 65],
                pexp[:, off:QC],
                start=(kt == 0), stop=(kt == nkt - 1))
            nc.tensor.matmul(
                oAB[:, QC + off:2 * QC],
                vp_sb[:, base + hB:base + hB + 128],
                pexp[:, QC + off:2 * QC],
                start=(kt == 0), stop=(kt == nkt - 1))

        def make_normalize(qc, p, oAB):
            """Two closures: (dn copies: DVE+ACT in parallel) and, >=2 steps
            later so the copies are done, (bc broadcast MM + divides)."""
            dn = dn_sb[p]
            hA, hB = 2 * p, 2 * p + 1

            def emit_dn():
                # dn row 64 <- 1/(head A denom + exp sink), row 0 <- head B
                # (DVE partition starts must be 32-aligned).
                with nc.allow_low_precision(reason="denom recip for PE bcast"):
                    nc.vector.tensor_scalar(
                        out=dn[64:65, :], in0=oAB[64:65, 0:QC],
                        scalar1=esk_sb[64:65, hA:hA + 1], scalar2=None,
                        op0=Alu.add)
                    nc.vector.tensor_scalar(
                        out=dn[0:1, :], in0=oAB[0:1, QC:2 * QC],
                        scalar1=esk_sb[0:1, hB:hB + 1], scalar2=None,
                        op0=Alu.add)
                    nc.vector.reciprocal(dn[64:65, :], dn[64:65, :])
                    nc.vector.reciprocal(dn[0:1, :], dn[0:1, :])

            def emit_bcdiv():
                bc = ps_p.tile([128, 2 * QC], f32, tag="ps")
                nc.tensor.matmul(
                    bc[:, 0:QC], ind2_sb[:, :], dn[:, :], start=True, stop=True)
                with nc.allow_low_precision(reason="softmax denom scale"):
                    # DVE reads at most one PSUM operand: land the broadcast
                    # 1/denoms back into dn (SBUF; its rows were consumed by
                    # the bc matmul above), then multiply PSUM * SBUF.
                    nc.vector.tensor_copy(dn[:, :], bc[:, 0:QC])
                    nc.vector.tensor_mul(
                        at_sb[0:64, p * T + qc * QC:p * T + qc * QC + QC],
                        oAB[0:64, 0:QC], dn[0:64, :])
                    nc.vector.tensor_mul(
                        at_sb[64:128, p * T + qc * QC:p * T + qc * QC + QC],
                        oAB[64:128, QC:2 * QC], dn[64:128, :])
            return emit_dn, emit_bcdiv

        def make_wout(qc, cop):
            def emit():
                ps = ps_p.tile([128, 2 * QC], f32, tag="ps")
                for half in range(2):
                    co = cop * 2 + half
                    for j in range(2):
                        nc.tensor.matmul(
                            ps[:, half * QC:(half + 1) * QC],
                            wo_sb[:, j * C + co * 128:j * C + (co + 1) * 128],
                            at_sb[:, j * T + qc * QC:j * T + qc * QC + QC],
                            start=(j == 0), stop=(j == 1))
                yt = y_p.tile([128, 2 * QC], bf16, tag="y")
                with nc.allow_low_precision(reason="bf16 partial writeback"):
                    nc.vector.tensor_copy(yt[:, :], ps[:, :])
                nc.sync.dma_start(
                    yt_v[:, cop * 2:cop * 2 + 2, qc * QC:qc * QC + QC],
                    yt[:, :].rearrange("p (n m) -> p n m", m=QC))
            return emit

        # ---- the pipelined body ----
        for _ in range(reps):
            # stage A: projections for chunk 0 / qc 0 (direct, DMA-gated;
            # ordered to match DMA arrival: wk, xt-c0, wv, wq)
            proj_qk_chunk(wk_sb, kt_view, 0)()
            proj_v_pair(0, 0)()
            proj_v_pair(0, 1)()
            proj_qk_chunk(wq_sb, qt_view, 0)()

            pend_norm = []   # closures from previous qc
            pend_wout = []
            prev = None      # (qc, p, kt, nkt, oAB, pexp, off)
            oABs = {}

            for qc in range(NQ):
                nkt = 4 * qc + 4
                # fillers: (min_step, closure), popped in list order once
                # min_step is reached. K/V projections for chunk qc pop at
                # the START of qc itself (diag tiles that need them are at
                # step >= 4*qc); Q^T for qc+1 pops anywhere in qc. norm dn
                # pops >=2 steps before its bc+div; wout after the divides.
                fillers = []
                if qc >= 2:
                    fillers.append((0, proj_qk_chunk(wk_sb, kt_view, qc)))
                if pend_norm:
                    dn_f, bcdiv_f = pend_norm
                    fillers.append((1, dn_f))
                if qc >= 2:
                    fillers.append((1, proj_v_pair(qc, 0)))
                if pend_norm:
                    fillers.append((3, bcdiv_f))
                if qc >= 2:
                    fillers.append((2, proj_v_pair(qc, 1)))
                fillers += [(4, f) for f in pend_wout]
                pend_norm, pend_wout = [], []
                if qc == 0:
                    fillers.append((0, proj_qk_chunk(wk_sb, kt_view, 1)))
                    fillers.append((2, proj_v_pair(1, 0)))
                    fillers.append((3, proj_v_pair(1, 1)))
                if qc + 1 < NQ:
                    fillers.append((2, proj_qk_chunk(wq_sb, qt_view, qc + 1)))

                steps = 2 * nkt
                popped = 0
                nf_static = len(fillers) + 2   # +2: norm(qc,p0) added mid-loop
                step = 0
                for p in range(2):
                    oABs[p] = o_p.tile([128, 2 * QC], f32, tag="o", name="oAB")
                    for kt in range(nkt):
                        cur = emit_scores(qc, p, kt)
                        # pop fillers: spread evenly, respect min_step
                        while (fillers
                               and popped * steps < (step + 1) * nf_static
                               and fillers[0][0] <= step):
                            fillers.pop(0)[1]()
                            popped += 1
                        if prev is not None:
                            emit_pv(*prev)
                        prev = (qc, p, kt, nkt, oABs[p], cur[0], cur[1])
                        step += 1
                    if p == 0:
                        dn_f, bcdiv_f = make_normalize(qc, 0, oABs[0])
                        fillers.append((nkt + 1, dn_f))
                        fillers.append((nkt + 3, bcdiv_f))
                for _, f in fillers:
                    f()
                pend_norm = make_normalize(qc, 1, oABs[1])
                pend_wout = [make_wout(qc, cop) for cop in range(NCC // 2)]

            emit_pv(*prev)
            prev = None
            dn_f, bcdiv_f = pend_norm
            dn_f()
            bcdiv_f()
            for f in pend_wout:
                f()

    nc.compile()
    return nc


def make_consts():
    import ml_dtypes
    k = np.arange(128)[:, None]
    j = np.arange(128)[None, :]
    tri = (j >= k).astype(np.float32).astype(ml_dtypes.bfloat16)
    vpc = np.zeros((128, 65), dtype=np.float32)
    vpc[:, 0:2] = 1.0
    vpc = np.tile(vpc, (1, NKT))
    ind2 = np.zeros((128, 128), dtype=np.float32)
    ind2[64, 0:64] = 1.0   # head A 1/denom (dn row 64) -> rows 0-63
    ind2[0, 64:128] = 1.0  # head B 1/denom (dn row 0) -> rows 64-127
    return tri, vpc, ind2


def shard_inputs(x, W_Q, W_K, W_V, W_out, sink):
    tri, vpc, ind2 = make_consts()
    in_maps = []
    for c in range(NCORES):
        b, g = divmod(c, G)
        cols = slice(g * DH, (g + 1) * DH)
        in_maps.append({
            "xt": np.ascontiguousarray(x[b].T),
            "wq": np.ascontiguousarray(W_Q[:, cols]),
            "wk": np.ascontiguousarray(W_K[:, cols]),
            "wv": np.ascontiguousarray(W_V[:, cols]),
            "wo": np.ascontiguousarray(W_out[cols, :]),
            "sk": np.ascontiguousarray(sink[g * G:(g + 1) * G][None, :]),
            "tri": tri,
            "vpc": vpc,
            "ind2": ind2,
            "dz": np.zeros((128, QC), dtype=np.float32),
        })
    return in_maps


def gather_outputs(results):
    out = np.zeros((B, T, C), dtype=np.float32)
    for b in range(B):
        acc = np.zeros((C, T), dtype=np.float32)
        for g in range(G):
            acc += np.asarray(results[b * G + g]["yt"], dtype=np.float32)
        out[b] = acc.T
    return out


_CACHE = {}


def _get_program():
    if "nc" not in _CACHE:
        _CACHE["nc"] = build_program(reps=1)
    return _CACHE["nc"]


def kernel(x, W_Q, W_K, W_V, W_out, sink):
    from concourse.bass_utils import run_bass_kernel_spmd

    x = np.asarray(x, dtype=np.float32)
    W_Q = np.asarray(W_Q, dtype=np.float32)
    W_K = np.asarray(W_K, dtype=np.float32)
    W_V = np.asarray(W_V, dtype=np.float32)
    W_out = np.asarray(W_out, dtype=np.float32)
    sink = np.asarray(sink, dtype=np.float32)

    nc = _get_program()
    in_maps = shard_inputs(x, W_Q, W_K, W_V, W_out, sink)
    res = run_bass_kernel_spmd(nc, in_maps, core_ids=list(range(NCORES)))
    return gather_outputs(res.results)


# revision 34
# speedup vs baseline: 1.1533x; 1.1533x over previous
"""Trainium2 Bass kernel: 16-head causal attention with sink logit.

Contract: kernel(**inputs) takes the FULL inputs of the reference
(x [2,2048,1024], W_Q/W_K/W_V/W_out [1024,1024], sink [16]) and returns
the FULL output [2,2048,1024], running on 8 NeuronCores.

Sharding: core c = b*4 + g handles batch b and heads [4g, 4g+4).
Each core computes yT_partial [1024, 2048] = W_out_slice^T @ attn^T;
host sums the 4 partials per batch and transposes.

v2 structure: single software-pipelined stream. Projections (K^T, V,
Q^T per 512-token chunk), W_out and normalize blocks are emitted as
"filler" closures popped between attention tile steps so the PE stays
busy while the scalar engine runs exp. Diagonal score tiles are
narrowed to the valid q-range; only the 128x128 triangle block is
masked. Denominators are broadcast raw via one PE matmul and the
normalization is a DVE divide reading PSUM directly.
"""

import sys
import numpy as np

if "/opt/trn_rl_repo" not in sys.path:
    sys.path.insert(0, "/opt/trn_rl_repo")

B, T, C = 2, 2048, 1024
H, D = 16, 64
G = 4                # heads per core
DH = G * D           # 256 head-dims per core
NCORES = 8
QC = 512             # q chunk (matmul moving free dim)
NQ = T // QC         # 4
NKT = T // 128       # 16 k-tiles
NCC = C // 128       # 8 contraction chunks over C
SCALE = 1.0 / float(np.sqrt(D))

# vp_sb per-kt slot layout (386 cols per kt):
#   head0 (even): [V(64) | one]            off 0,   width 65,  denom row 64
#   head1 (odd):  [one | zeros(63) | V(64)] off 65,  width 128, denom row 0
#   head2 (even): [V(64) | one]            off 193, width 65,  denom row 64
#   head3 (odd):  [one | zeros(63) | V(64)] off 258, width 128, denom row 0
VP_W = 386
VP_OFF = [0, 65, 193, 258]
VSEG = [(0, 0, 64), (64, 129, 128), (192, 322, 64)]  # (psum off, vp off, width)


def build_program(reps=1):
    """Build the per-core Bass program. reps>1 repeats the compute body
    (same inputs -> same outputs) for differential wall-clock timing."""
    from contextlib import ExitStack

    import concourse.bass as bass
    import concourse.tile as tile
    from concourse import bacc, mybir

    f32 = mybir.dt.float32
    f32r = mybir.dt.float32r
    bf16 = mybir.dt.bfloat16
    AF = mybir.ActivationFunctionType
    Alu = mybir.AluOpType

    nc = bacc.Bacc("TRN2", target_bir_lowering=False, debug=False)

    xt_d = nc.dram_tensor("xt", [C, T], f32r, kind="ExternalInput").ap()
    wq_d = nc.dram_tensor("wq", [C, DH], f32r, kind="ExternalInput").ap()
    wk_d = nc.dram_tensor("wk", [C, DH], f32r, kind="ExternalInput").ap()
    wv_d = nc.dram_tensor("wv", [C, DH], f32r, kind="ExternalInput").ap()
    wo_d = nc.dram_tensor("wo", [DH, C], f32r, kind="ExternalInput").ap()
    sk_d = nc.dram_tensor("sk", [1, G], f32, kind="ExternalInput").ap()
    tri_d = nc.dram_tensor("tri", [128, 128], bf16, kind="ExternalInput").ap()
    vpc_d = nc.dram_tensor("vpc", [128, NKT * 65], f32r, kind="ExternalInput").ap()
    ind2_d = nc.dram_tensor("ind2", [128, 128], f32r, kind="ExternalInput").ap()
    dz_d = nc.dram_tensor("dz", [128, QC], f32r, kind="ExternalInput").ap()
    yt_d = nc.dram_tensor("yt", [C, T], bf16, kind="ExternalOutput").ap()

    xt_v = xt_d.rearrange("(n p) m -> p n m", p=128)   # [128, 8, 2048]
    wq_v = wq_d.rearrange("(n p) m -> p n m", p=128)   # [128, 8, 256]
    wk_v = wk_d.rearrange("(n p) m -> p n m", p=128)
    wv_v = wv_d.rearrange("(n p) m -> p n m", p=128)
    wo_v = wo_d.rearrange("(n p) m -> p n m", p=128)   # [128, 2, 1024]
    yt_v = yt_d.rearrange("(n p) m -> p n m", p=128)   # [128, 8, 2048]

    with tile.TileContext(nc) as tc, ExitStack() as ctx:
        P = lambda name, bufs: ctx.enter_context(tc.tile_pool(name=name, bufs=bufs))
        const_p = P("const", 1)
        big_p = P("big", 1)
        p_p = P("p", 4)
        y_p = P("y", 2)
        ps_p = ctx.enter_context(tc.tile_pool(name="ps", bufs=2, space="PSUM"))
        o_p = ctx.enter_context(tc.tile_pool(name="o", bufs=2, space="PSUM"))

        # ---- persistent SBUF tensors ----
        xt_sb = big_p.tile([128, NCC * T], f32r, tag="xt")           # 64KB/part
        wq_sb = big_p.tile([128, NCC * DH], f32r, tag="wq")
        wk_sb = big_p.tile([128, NCC * DH], f32r, tag="wk")
        wv_sb = big_p.tile([128, NCC * DH], f32r, tag="wv")
        wo_sb = big_p.tile([128, 2 * C], f32r, tag="wo")
        qt_sb = big_p.tile([128, 2 * T], f32r, tag="qt")
        kt_sb = big_p.tile([128, 2 * T], f32r, tag="kt")
        vp_sb = big_p.tile([128, NKT * VP_W], f32r, tag="vp")
        at_sb = big_p.tile([128, 2 * T], f32r, tag="at")             # attn^T normalized
        tri_sb = const_p.tile([128, 128], bf16, tag="tri")
        ind2_sb = const_p.tile([128, 128], f32r, tag="ind2")
        skr_sb = const_p.tile([128, G], f32, tag="skr")
        esk_sb = const_p.tile([128, G], f32, tag="esk")
        dn0_sb = const_p.tile([128, QC], f32r, tag="dn0")
        dn1_sb = const_p.tile([128, QC], f32r, tag="dn1")
        dn_sb = [dn0_sb, dn1_sb]

        xt_view = xt_sb[:].rearrange("p (n m) -> p n m", m=T)        # [128,8,2048]
        qt_view = qt_sb[:].rearrange("p (n m) -> p n m", m=T)        # [128,2,2048]
        kt_view = kt_sb[:].rearrange("p (n m) -> p n m", m=T)
        vp_view = vp_sb[:].rearrange("p (k w) -> p k w", w=VP_W)     # [128,16,386]

        # ---- phase 0: loads + constants (outside reps; ordered for the
        # pipeline: consts, wk, xt chunk0, wv, wq, xt c1, c2, wo, c3) ----
        nc.sync.dma_start(tri_sb[:, :], tri_d[:, :])
        nc.sync.dma_start(ind2_sb[:, :], ind2_d[:, :])
        nc.sync.dma_start(skr_sb[0:1, :], sk_d[:, :])
        nc.sync.dma_start(skr_sb[64:65, :], sk_d[:, :])
        nc.scalar.activation(esk_sb[0:1, :], skr_sb[0:1, :], AF.Exp)
        nc.scalar.activation(esk_sb[64:65, :], skr_sb[64:65, :], AF.Exp)
        for dn in dn_sb:
            nc.sync.dma_start(dn[:, :], dz_d[:, :])
        vpc_view = vpc_d.rearrange("p (k w) -> p k w", w=65)
        nc.sync.dma_start(vp_view[:, :, 64:129], vpc_view[:, :, :])
        nc.sync.dma_start(vp_view[:, :, 257:322], vpc_view[:, :, :])

        def dma_xt_chunk(c):
            # per-ci pieces so the first projection matmuls can chase the
            # DMA stream instead of waiting for the whole 2MB chunk
            for ci in range(NCC):
                nc.sync.dma_start(
                    xt_view[:, ci, c * QC:(c + 1) * QC],
                    xt_v[:, ci, c * QC:(c + 1) * QC])

        nc.sync.dma_start(
            wk_sb[:].rearrange("p (n m) -> p n m", m=DH), wk_v[:, :, :])
        dma_xt_chunk(0)
        nc.sync.dma_start(
            wq_sb[:].rearrange("p (n m) -> p n m", m=DH), wq_v[:, :, :])
        nc.sync.dma_start(
            wv_sb[:].rearrange("p (n m) -> p n m", m=DH), wv_v[:, :, :])
        dma_xt_chunk(1)
        dma_xt_chunk(2)
        nc.sync.dma_start(
            wo_sb[:].rearrange("p (n m) -> p n m", m=C), wo_v[:, :, :])
        dma_xt_chunk(3)

        # ---- emission helpers ----

        def proj_qk_chunk(w_sb, dst_view, cidx):
            """Project one 512-token chunk of Q^T or K^T: [2*128 d, 512 t]."""
            def emit():
                ps = ps_p.tile([128, 2 * QC], f32, tag="ps")
                for mt in range(2):
                    for ci in range(NCC):
                        nc.tensor.matmul(
                            ps[:, mt * QC:(mt + 1) * QC],
                            w_sb[:, ci * DH + mt * 128: ci * DH + (mt + 1) * 128],
                            xt_view[:, ci, cidx * QC:(cidx + 1) * QC],
                            start=(ci == 0), stop=(ci == NCC - 1))
                nc.vector.tensor_copy(
                    dst_view[:, :, cidx * QC:(cidx + 1) * QC],
                    ps[:].rearrange("p (n m) -> p n m", m=QC))
            return emit

        def proj_v_pair(cidx, pair):
            """V natural [t, d] for 2 of the 4 128-token tiles of chunk cidx."""
            def emit():
                ps = ps_p.tile([128, 2 * QC], f32, tag="ps")
                for s in range(2):
                    tt = cidx * 4 + pair * 2 + s
                    for ci in range(NCC):
                        nc.tensor.matmul(
                            ps[:, s * DH:(s + 1) * DH],
                            xt_view[:, ci, tt * 128:(tt + 1) * 128],
                            wv_sb[:, ci * DH:(ci + 1) * DH],
                            start=(ci == 0), stop=(ci == NCC - 1))
                ps_v = ps[:].rearrange("p (n m) -> p n m", m=DH)
                tt0 = cidx * 4 + pair * 2
                for so, do, w in VSEG:
                    nc.vector.tensor_copy(
                        vp_view[:, tt0:tt0 + 2, do:do + w],
                        ps_v[:, 0:2, so:so + w])
            return emit

        def emit_scores(qc, p, kt):
            """Scores + exp (+ triangle mask on diag) for one (p, kt) tile.
            Returns (pexp, off) for the later PV step."""
            diag = kt - 4 * qc
            off = 0 if diag < 0 else 128 * diag
            ps = ps_p.tile([128, 2 * QC], f32, tag="ps")
            nc.tensor.matmul(
                ps[:, off:QC],
                kt_sb[0:64, p * T + kt * 128:p * T + (kt + 1) * 128],
                qt_sb[0:64, p * T + qc * QC + off:p * T + qc * QC + QC],
                start=True, stop=True)
            nc.tensor.matmul(
                ps[:, QC + off:2 * QC],
                kt_sb[64:128, p * T + kt * 128:p * T + (kt + 1) * 128],
                qt_sb[64:128, p * T + qc * QC + off:p * T + qc * QC + QC],
                start=True, stop=True)
            pexp = p_p.tile([128, 2 * QC], f32r, tag="p")
            ps_h = ps[:].rearrange("p (n m) -> p n m", m=QC)
            pe_h = pexp[:].rearrange("p (n m) -> p n m", m=QC)
            nc.scalar.activation(
                pe_h[:, :, off:QC], ps_h[:, :, off:QC], AF.Exp, scale=SCALE)
            if diag >= 0:
                with nc.allow_low_precision(reason="0/1 mask mult"):
                    nc.gpsimd.tensor_mul(
                        pexp[:, off:off + 128], pexp[:, off:off + 128],
                        tri_sb[:, :])
                    nc.vector.tensor_mul(
                        pexp[:, QC + off:QC + off + 128],
                        pexp[:, QC + off:QC + off + 128], tri_sb[:, :])
            return pexp, off

        def emit_pv(qc, p, kt, nkt, oAB, pexp, off):
            base = kt * VP_W
            hA = VP_OFF[2 * p]
            hB = VP_OFF[2 * p + 1]
            nc.tensor.matmul(
                oAB[0:65, off:QC],
                vp_sb[:, base + hA:base + hA + 65],
                pexp[:, off:QC],
                start=(kt == 0), stop=(kt == nkt - 1))
            nc.tensor.matmul(
                oAB[:, QC + off:2 * QC],
                vp_sb[:, base + hB:base + hB + 128],
                pexp[:, QC + off:2 * QC],
                start=(kt == 0), stop=(kt == nkt - 1))

        def make_normalize(qc, p, oAB, tail=False):
            """Two closures: (dn denom-row copies) and, >=2 steps later so
            the copies are done, (bc broadcast MM + recip + muls). At the
            tail ACT is idle, so run one row copy there in parallel."""
            dn = dn_sb[p]
            hA, hB = 2 * p, 2 * p + 1

            def emit_dn():
                # dn row 64 <- head A denom + exp sink, row 0 <- head B
                # (DVE partition starts must be 32-aligned).
                with nc.allow_low_precision(reason="denom rows to sbuf"):
                    nc.vector.tensor_scalar(
                        out=dn[64:65, :], in0=oAB[64:65, 0:QC],
                        scalar1=esk_sb[64:65, hA:hA + 1], scalar2=None,
                        op0=Alu.add)
                    if tail:
                        nc.scalar.activation(
                            dn[0:1, :], oAB[0:1, QC:2 * QC], AF.Identity,
                            bias=esk_sb[0:1, hB:hB + 1])
                        return
                    nc.vector.tensor_scalar(
                        out=dn[0:1, :], in0=oAB[0:1, QC:2 * QC],
                        scalar1=esk_sb[0:1, hB:hB + 1], scalar2=None,
                        op0=Alu.add)

            def emit_bcdiv():
                bc = ps_p.tile([128, 2 * QC], f32, tag="ps")
                nc.tensor.matmul(
                    bc[:, 0:QC], ind2_sb[:, :], dn[:, :], start=True, stop=True)
                with nc.allow_low_precision(reason="softmax denom scale"):
                    # one wide reciprocal PSUM->SBUF lands 1/denom back into
                    # dn (its rows were consumed by the bc matmul above);
                    # then multiply PSUM * SBUF (DVE allows one PSUM operand).
                    nc.vector.reciprocal(dn[:, :], bc[:, 0:QC])
                    nc.vector.tensor_mul(
                        at_sb[0:64, p * T + qc * QC:p * T + qc * QC + QC],
                        oAB[0:64, 0:QC], dn[0:64, :])
                    nc.vector.tensor_mul(
                        at_sb[64:128, p * T + qc * QC:p * T + qc * QC + QC],
                        oAB[64:128, QC:2 * QC], dn[64:128, :])
            return emit_dn, emit_bcdiv

        def make_wout(qc, cop):
            def emit():
                ps = ps_p.tile([128, 2 * QC], f32, tag="ps")
                for half in range(2):
                    co = cop * 2 + half
                    for j in range(2):
                        nc.tensor.matmul(
                            ps[:, half * QC:(half + 1) * QC],
                            wo_sb[:, j * C + co * 128:j * C + (co + 1) * 128],
                            at_sb[:, j * T + qc * QC:j * T + qc * QC + QC],
                            start=(j == 0), stop=(j == 1))
                yt = y_p.tile([128, 2 * QC], bf16, tag="y")
                with nc.allow_low_precision(reason="bf16 partial writeback"):
                    nc.vector.tensor_copy(yt[:, :], ps[:, :])
                nc.sync.dma_start(
                    yt_v[:, cop * 2:cop * 2 + 2, qc * QC:qc * QC + QC],
                    yt[:, :].rearrange("p (n m) -> p n m", m=QC))
            return emit

        # ---- the pipelined body ----
        for _ in range(reps):
            # stage A: projections for chunk 0 / qc 0 (direct, DMA-gated;
            # ordered to match DMA arrival: wk, xt-c0, wq, wv)
            proj_qk_chunk(wk_sb, kt_view, 0)()
            proj_qk_chunk(wq_sb, qt_view, 0)()
            proj_v_pair(0, 0)()
            proj_v_pair(0, 1)()

            pend_norm = []   # closures from previous qc
            pend_wout = []
            pvq = []         # pending PV work, depth 2: PE consumes the
                             # tile from two steps back so exp+mask sems
                             # are pre-satisfied when the PE reaches it
            oABs = {}

            for qc in range(NQ):
                nkt = 4 * qc + 4
                # fillers: (min_step, closure), popped in list order once
                # min_step is reached. K/V projections for chunk qc pop at
                # the START of qc itself (diag tiles that need them are at
                # step >= 4*qc); Q^T for qc+1 pops anywhere in qc. norm dn
                # pops >=2 steps before its bc+div; wout after the divides.
                # NOTE: with the lag-2 PV queue, the last PV of (qc-1, p1)
                # is EMITTED during step 1; consumers of its oAB must pop at
                # step >= 2 or they miss the dependency at emission time.
                fillers = []
                if qc >= 2:
                    fillers.append((0, proj_qk_chunk(wk_sb, kt_view, qc)))
                if pend_norm:
                    dn_f, bcdiv_f = pend_norm
                    fillers.append((2, dn_f))
                if qc >= 2:
                    fillers.append((1, proj_v_pair(qc, 0)))
                if pend_norm:
                    fillers.append((4, bcdiv_f))
                if qc >= 2:
                    fillers.append((2, proj_v_pair(qc, 1)))
                fillers += [(5, f) for f in pend_wout]
                pend_norm, pend_wout = [], []
                if qc == 0:
                    fillers.append((0, proj_qk_chunk(wk_sb, kt_view, 1)))
                    fillers.append((2, proj_v_pair(1, 0)))
                    fillers.append((3, proj_v_pair(1, 1)))
                if qc + 1 < NQ:
                    fillers.append((2, proj_qk_chunk(wq_sb, qt_view, qc + 1)))

                steps = 2 * nkt
                popped = 0
                nf_static = len(fillers) + 2   # +2: norm(qc,p0) added mid-loop
                step = 0
                for p in range(2):
                    oABs[p] = o_p.tile([128, 2 * QC], f32, tag="o", name="oAB")
                    for kt in range(nkt):
                        cur = emit_scores(qc, p, kt)
                        # pop fillers: spread evenly, respect min_step
                        while (fillers
                               and popped * steps < (step + 1) * nf_static
                               and fillers[0][0] <= step):
                            fillers.pop(0)[1]()
                            popped += 1
                        pvq.append((qc, p, kt, nkt, oABs[p], cur[0], cur[1]))
                        if len(pvq) > 2:
                            emit_pv(*pvq.pop(0))
                        step += 1
                    if p == 0:
                        dn_f, bcdiv_f = make_normalize(qc, 0, oABs[0])
                        fillers.append((nkt + 3, dn_f))
                        fillers.append((nkt + 5, bcdiv_f))
                for _, f in fillers:
                    f()
                pend_norm = make_normalize(qc, 1, oABs[1], tail=(qc == NQ - 1))
                pend_wout = [make_wout(qc, cop) for cop in range(NCC // 2)]

            while pvq:
                emit_pv(*pvq.pop(0))
            dn_f, bcdiv_f = pend_norm
            dn_f()
            bcdiv_f()
            for f in pend_wout:
                f()

    nc.compile()
    return nc


def make_consts():
    import ml_dtypes
    k = np.arange(128)[:, None]
    j = np.arange(128)[None, :]
    tri = (j >= k).astype(np.float32).astype(ml_dtypes.bfloat16)
    vpc = np.zeros((128, 65), dtype=np.float32)
    vpc[:, 0:2] = 1.0
    vpc = np.tile(vpc, (1, NKT))
    ind2 = np.zeros((128, 128), dtype=np.float32)
    ind2[64, 0:64] = 1.0   # head A 1/denom (dn row 64) -> rows 0-63
    ind2[0, 64:128] = 1.0  # head B 1/denom (dn row 0) -> rows 64-127
    return tri, vpc, ind2


def shard_inputs(x, W_Q, W_K, W_V, W_out, sink):
    tri, vpc, ind2 = make_consts()
    in_maps = []
    for c in range(NCORES):
        b, g = divmod(c, G)
        cols = slice(g * DH, (g + 1) * DH)
        in_maps.append({
            "xt": np.ascontiguousarray(x[b].T),
            "wq": np.ascontiguousarray(W_Q[:, cols]),
            "wk": np.ascontiguousarray(W_K[:, cols]),
            "wv": np.ascontiguousarray(W_V[:, cols]),
            "wo": np.ascontiguousarray(W_out[cols, :]),
            "sk": np.ascontiguousarray(sink[g * G:(g + 1) * G][None, :]),
            "tri": tri,
            "vpc": vpc,
            "ind2": ind2,
            "dz": np.zeros((128, QC), dtype=np.float32),
        })
    return in_maps


def gather_outputs(results):
    out = np.zeros((B, T, C), dtype=np.float32)
    for b in range(B):
        acc = np.zeros((C, T), dtype=np.float32)
        for g in range(G):
            acc += np.asarray(results[b * G + g]["yt"], dtype=np.float32)
        out[b] = acc.T
    return out


_CACHE = {}


def _get_program():
    if "nc" not in _CACHE:
        _CACHE["nc"] = build_program(reps=1)
    return _CACHE["nc"]


def kernel(x, W_Q, W_K, W_V, W_out, sink):
    from concourse.bass_utils import run_bass_kernel_spmd

    x = np.asarray(x, dtype=np.float32)
    W_Q = np.asarray(W_Q, dtype=np.float32)
    W_K = np.asarray(W_K, dtype=np.float32)
    W_V = np.asarray(W_V, dtype=np.float32)
    W_out = np.asarray(W_out, dtype=np.float32)
    sink = np.asarray(sink, dtype=np.float32)

    nc = _get_program()
    in_maps = shard_inputs(x, W_Q, W_K, W_V, W_out, sink)
    res = run_bass_kernel_spmd(nc, in_maps, core_ids=list(range(NCORES)))
    return gather_outputs(res.results)
